# revision 16
# baseline (speedup 1.0000x reference)
"""Trainium2 Bass kernel, v2: top-k COMPACTED mixers.

Same agent/rank machinery as v1 (full-N, fp32), but the mixers run on
a compacted token set: the top-k tokens (k per-sample dynamic, padded
to a compile-time multiple of 128 chosen from a cheap host estimate)
are gathered via indirect DMA using a device-computed rank->token
inverse permutation. Attention pads are masked with a slot-index bias;
the edge highlighting is evaluated in token-major form (per-token
scalars become per-partition scalars) and scattered back over a
token-major base canvas with out-of-quota slots redirected to a trash
row. The host transposes the token-major output back to (C, H, W).
"""

import numpy as np


DIM = 256
N = 1024
HID = 512
NH = 4
B = 8

_CACHE = {}


def _install_compat():
    """Environment shims: walrus here accepts at most ONE sync-wait per
    instruction; Tile's kernel-tail drain aggregates many -> split them
    onto single-wait DVE nops. Also make upload_artifacts local-only."""
    import concourse.mybir as mybir
    import concourse.tile as tile
    from concourse.vector_clock import ScopedClock
    from concourse import bass_utils

    if not getattr(tile.TileContext, "_drain_patched", False):

        def _patched(self, tick_clock, wait_clock):
            nc = self.nc
            drain_inst = nc.sync.drain()
            wait_clock.add_sem_waits(
                drain_inst.ins, ScopedClock({None: tick_clock.global_clock})
            )
            si = drain_inst.ins.sync_info
            waits = list(si.on_wait)
            if len(waits) > 1:
                drain_inst.ins.sync_info = mybir.SyncInfo(
                    on_wait=[], on_update=list(si.on_update)
                )
                for i in range(len(waits)):
                    nop = nc.vector.engine_nop()
                    nop.ins.sync_info = mybir.SyncInfo(
                        on_wait=waits[i : i + 1], on_update=[]
                    )
            nc.all_engine_barrier()
            assert self.sems is not None
            popped = nc._tile_sem_poison_stack.pop()
            assert popped is self._sem_poison
            nc.clear_and_free_semaphores(list(self.sems.allocated().values()))
            nc.all_engine_barrier()

        tile.TileContext._drain_and_barrier = _patched
        tile.TileContext._drain_patched = True

    bass_utils.upload_artifacts = lambda tmpdir: str(tmpdir)


def _to_bf16(a):
    import ml_dtypes

    return np.asarray(a, dtype=np.float32).astype(ml_dtypes.bfloat16)


def _prep_inputs_v1(inputs):
    """Host-side packing: per-core activation tensors + replicated
    (layout-transposed, LN-folded) weights."""
    f = {k: np.asarray(v, dtype=np.float32) for k, v in inputs.items()}
    shared = {}

    ascale = (f["bn_g"] / np.float32(np.sqrt(1.0 + 1e-5))).astype(np.float32)
    abias = (f["ab1"] * ascale + f["bn_b"]).astype(np.float32)
    shared["aw1T"] = np.ascontiguousarray(f["aw1"].T).astype(np.float32)
    shared["agsc"] = ascale
    shared["agbi"] = abias
    shared["aw2T"] = np.ascontiguousarray(f["aw2"].reshape(1, HID).T).reshape(HID)
    scal = np.zeros((1, 8), np.float32)
    scal[0, 0] = f["ab2"].reshape(-1)[0]
    scal[0, 1] = f["hb2"].reshape(-1)[0]
    scal[0, 2] = np.float32(f["highlight_scale"])
    scal[0, 3] = np.float32(-0.5)
    scal[0, 4] = np.float32(1e-5)
    shared["scal"] = scal
    hrow = np.zeros((1, 48), np.float32)
    hrow[0, 0:16] = f["hw1"].reshape(16) / np.float32(N)
    hrow[0, 16:32] = f["hb1"].reshape(16)
    hrow[0, 32:48] = f["hw2"].reshape(16)
    shared["hrow"] = hrow

    for p in ("ir", "vis"):
        ln_g = f[p + "_ln_g"]
        ln_b = f[p + "_ln_b"]
        qkv_w = f[p + "_qkv_w"]
        qkv_b = f[p + "_qkv_b"]
        out_w = f[p + "_out_w"]
        out_b = f[p + "_out_b"]
        w1 = f[p + "_ffn_w1"]
        b1 = f[p + "_ffn_b1"]
        w2 = f[p + "_ffn_w2"]
        b2 = f[p + "_ffn_b2"]
        qkvT_eff = (qkv_w * ln_g[None, :]).T  # [256, 768]
        qkvb_eff = qkv_b + qkv_w @ ln_b
        w1T_eff = (w1 * ln_g[None, :]).T  # [256, 1024]
        b1_eff = b1 + w1 @ ln_b
        bv = qkvb_eff[2 * DIM :]
        outb_eff = out_b + out_w @ bv
        shared[p + "_qkvT"] = _to_bf16(qkvT_eff)
        shared[p + "_qb"] = qkvb_eff[:DIM].astype(np.float32)
        shared[p + "_outT"] = _to_bf16(out_w.T)
        shared[p + "_outb"] = outb_eff.astype(np.float32)
        shared[p + "_w1T"] = _to_bf16(w1T_eff)
        shared[p + "_b1"] = b1_eff.astype(np.float32)
        shared[p + "_w2T"] = _to_bf16(w2.T)
        shared[p + "_b2"] = b2.astype(np.float32)

    per_core = []
    fir = f["f_ir"].reshape(B, DIM, N)
    fvis = f["f_vis"].reshape(B, DIM, N)
    for b in range(B):
        m = dict(shared)
        m["fir"] = np.ascontiguousarray(fir[b])
        m["fvis"] = np.ascontiguousarray(fvis[b])
        per_core.append(m)
    return per_core




def _split_multi_waits(nc):
    """This container's walrus accepts only ONE sync-wait per
    instruction: hoist extra waits onto same-engine nop carriers
    inserted immediately before the instruction."""
    import concourse.mybir as mybir

    for f in nc.m.functions:
        for bb in f.blocks:
            insts = list(bb.instructions)
            rebuilt = []
            changed = False
            for inst in insts:
                si = inst.sync_info
                waits = list(si.on_wait) if si is not None else []
                if len(waits) > 1:
                    changed = True
                    eng = inst.engine
                    for wx in waits[:-1]:
                        wrap = nc.engines[eng].nop(nofuse=True)
                        mi = wrap.ins
                        # remove from wherever add_instruction appended it
                        for f2 in nc.m.functions:
                            for bb2 in f2.blocks:
                                lst = list(bb2.instructions)
                                if lst and lst[-1] is mi:
                                    lst.pop()
                                    bb2.instructions = lst
                        mi.sync_info = mybir.SyncInfo(on_wait=[wx], on_update=[])
                        rebuilt.append(mi)
                    inst.sync_info = mybir.SyncInfo(
                        on_wait=[waits[-1]], on_update=list(si.on_update)
                    )
                rebuilt.append(inst)
            if changed:
                bb.instructions = rebuilt




def _host_k_estimate(inputs):
    f = {k: np.asarray(v, dtype=np.float32) for k, v in inputs.items()}
    x = np.concatenate([f["f_ir"], f["f_vis"]], axis=1).reshape(B, 2 * DIM, N)
    h = np.einsum("bcn,oc->bon", x, f["aw1"]) + f["ab1"][None, :, None]
    h = h / np.float32(np.sqrt(1.0 + 1e-5)) * f["bn_g"][None, :, None] + f["bn_b"][None, :, None]
    h = np.maximum(h, 0)
    z = np.einsum("bcn,oc->bon", h, f["aw2"]) + f["ab2"][None, :, None]
    w = 1.0 / (1.0 + np.exp(-z))
    score = np.abs(w - 0.5).reshape(B, N)
    gs = score.mean(axis=1, keepdims=True)
    t = np.maximum(gs @ f["hw1"].T + f["hb1"], 0)
    kr = 1.0 / (1.0 + np.exp(-(t @ f["hw2"].T + f["hb2"]))) * 0.8 + 0.1
    k = np.maximum(np.floor(N * kr[:, 0]).astype(np.int64), 64)
    return int(k.max())


def _prep_inputs(inputs):
    per_core = _prep_inputs_v1(inputs)
    f_ir = np.asarray(inputs["f_ir"], np.float32).reshape(B, DIM, N)
    f_vis = np.asarray(inputs["f_vis"], np.float32).reshape(B, DIM, N)
    for b in range(B):
        ftm = np.empty((N, 2 * DIM), np.float32)
        ftm[:, :DIM] = f_ir[b].T
        ftm[:, DIM:] = f_vis[b].T
        per_core[b]["ftm"] = ftm
    return per_core


def _build(nkc, debug=False):
    from contextlib import ExitStack

    import concourse.bass as bass
    import concourse.mybir as mybir
    import concourse.tile as tile
    from concourse.masks import make_identity

    f32 = mybir.dt.float32
    bf16 = mybir.dt.bfloat16
    i32 = mybir.dt.int32
    AF = mybir.ActivationFunctionType
    OP = mybir.AluOpType
    AX = mybir.AxisListType

    NC = nkc * 128  # compact token count (padded)

    def halves(n):
        out = []
        o = 0
        while o < n:
            w = min(512, n - o)
            out.append(slice(o, o + w))
            o += w
        return out

    HN = halves(N)
    HC = halves(NC)

    nc = bass.Bass("TRN2", target_bir_lowering=False, debug=False, enable_asserts=True)

    def din(name, shape, dt=f32):
        return nc.dram_tensor(name, shape, dt, kind="ExternalInput").ap()

    f32r_dt = mybir.dt.float32r
    fir_d = din("fir", [DIM, N], f32r_dt)
    fvis_d = din("fvis", [DIM, N], f32r_dt)
    ftm_d = din("ftm", [N, 2 * DIM])
    aw1T_d = din("aw1T", [2 * DIM, HID], f32r_dt)
    agsc_d = din("agsc", [HID])
    agbi_d = din("agbi", [HID])
    aw2T_d = din("aw2T", [HID], f32r_dt)
    scal_d = din("scal", [1, 8])
    hrow_d = din("hrow", [1, 48])
    mix_d = {}
    for p in ("ir", "vis"):
        mix_d[p] = dict(
            qkvT=din(p + "_qkvT", [DIM, 3 * DIM], bf16),
            qb=din(p + "_qb", [DIM]),
            outT=din(p + "_outT", [DIM, DIM], bf16),
            outb=din(p + "_outb", [DIM]),
            w1T=din(p + "_w1T", [DIM, 4 * DIM], bf16),
            b1=din(p + "_b1", [4 * DIM]),
            w2T=din(p + "_w2T", [4 * DIM, DIM], bf16),
            b2=din(p + "_b2", [DIM]),
        )

    out_d = nc.dram_tensor("out", [N + 1, DIM], f32, kind="ExternalOutput").ap()
    wsm_d = nc.dram_tensor("wsmeta", [N, 2], f32, kind="Internal").ap()
    dbg_d = {}
    if debug:
        for nm, shp in (
            ("d_mask", [1, N]),
            ("d_idx", [128, 8]),
            ("d_refir", [DIM, NC]),
            ("d_refvis", [DIM, NC]),
        ):
            dbg_d[nm] = nc.dram_tensor(nm, shp, f32, kind="ExternalOutput").ap()

    ctx = ExitStack()
    with tile.TileContext(nc) as tc:
        wp = ctx.enter_context(tc.tile_pool(name="wp", bufs=1))
        xp = ctx.enter_context(tc.tile_pool(name="xp", bufs=1))
        hp = ctx.enter_context(tc.tile_pool(name="hp", bufs=1))
        sp = ctx.enter_context(tc.tile_pool(name="sp", bufs=2))
        pps = ctx.enter_context(tc.tile_pool(name="pps", bufs=2, space="PSUM"))
        ppo = ctx.enter_context(tc.tile_pool(name="ppo", bufs=1, space="PSUM"))
        ppm = ctx.enter_context(tc.tile_pool(name="ppm", bufs=1, space="PSUM"))

        def col128(dram_vec, n, dt=f32):
            t = wp.tile([128, n], dt, tag=dram_vec.tensor.name)
            nc.gpsimd.dma_start(t[:], dram_vec.rearrange("(c p) -> p c", p=128))
            return t

        # ---- inputs first (agent path is the serial head of the kernel) ----
        fir_sb, fvis_sb = [], []
        for c in range(2):
            t = xp.tile([128, N], f32r_dt, tag=f"fir{c}", name=f"fir{c}")
            nc.sync.dma_start(t[:], fir_d[c * 128 : (c + 1) * 128, :])
            fir_sb.append(t)
            t = xp.tile([128, N], f32r_dt, tag=f"fvis{c}", name=f"fvis{c}")
            nc.sync.dma_start(t[:], fvis_d[c * 128 : (c + 1) * 128, :])
            fvis_sb.append(t)
        xcat = fir_sb + fvis_sb

        # ---- weights ----
        aw1T_sb = []
        for kc in range(4):
            t = wp.tile([128, HID], f32r_dt, tag=f"aw1T{kc}")
            nc.gpsimd.dma_start(t[:], aw1T_d[kc * 128 : (kc + 1) * 128, :])
            aw1T_sb.append(t)
        agsc_sb = col128(agsc_d, 4)
        agbi_sb = col128(agbi_d, 4)
        aw2c_sb = col128(aw2T_d, 4, f32r_dt)
        scal_sb = wp.tile([1, 8], f32, tag="scal")
        nc.gpsimd.dma_start(scal_sb[:], scal_d[:])
        hrow_sb = wp.tile([1, 48], f32, tag="hrow")
        nc.gpsimd.dma_start(hrow_sb[:], hrow_d[:])

        mix_w = {}
        for p in ("ir", "vis"):
            d = mix_d[p]
            w = {"qkvT": [], "outT": [], "w1T": [], "w2T": []}
            for c in range(2):
                t = wp.tile([128, 3 * DIM], bf16, tag=f"{p}qkvT{c}")
                nc.gpsimd.dma_start(t[:], d["qkvT"][c * 128 : (c + 1) * 128, :])
                w["qkvT"].append(t)
            w["qb"] = col128(d["qb"], 2)
            for c in range(2):
                t = wp.tile([128, DIM], bf16, tag=f"{p}outT{c}")
                nc.gpsimd.dma_start(t[:], d["outT"][c * 128 : (c + 1) * 128, :])
                w["outT"].append(t)
            w["outb"] = col128(d["outb"], 2)
            for c in range(2):
                t = wp.tile([128, 4 * DIM], bf16, tag=f"{p}w1T{c}")
                nc.sync.dma_start(t[:], d["w1T"][c * 128 : (c + 1) * 128, :])
                w["w1T"].append(t)
            w["b1"] = col128(d["b1"], 8)
            for kc in range(8):
                t = wp.tile([128, DIM], bf16, tag=f"{p}w2T{kc}")
                nc.sync.dma_start(t[:], d["w2T"][kc * 128 : (kc + 1) * 128, :])
                w["w2T"].append(t)
            w["b2"] = col128(d["b2"], 2)
            mix_w[p] = w

        ones_row = wp.tile([1, 128], f32, tag="ones_row")
        nc.vector.memset(ones_row[:], 1.0)
        ones_row_bf = wp.tile([1, 128], bf16, tag="ones_row_bf")
        nc.vector.memset(ones_row_bf[:], 1.0)
        ones_colb = wp.tile([128, 1], bf16, tag="ones_colb")
        nc.vector.memset(ones_colb[:], 1.0)
        ident = wp.tile([128, 128], f32, tag="ident")
        make_identity(nc, ident[:])
        ident_bf = wp.tile([64, 64], bf16, tag="ident_bf")
        make_identity(nc, ident_bf[:])
        ident_bf_hi = wp.tile([128, 64], bf16, tag="ident_bf_hi")
        nc.gpsimd.memset(ident_bf_hi[:], 0.0)
        make_identity(nc, ident_bf_hi[64:128, 0:64], nomemset=True)
        # slot iota [128, 8]: val[p, c] = c*128 + p
        iota_i = wp.tile([128, 8], i32, tag="iota_i")
        nc.gpsimd.iota(iota_i[:], pattern=[[128, 8]], base=0, channel_multiplier=1)
        iota_f = wp.tile([128, 8], f32, tag="iota_f")
        nc.vector.tensor_copy(iota_f[:], iota_i[:])
        iota_r = wp.tile([128, 8], f32r_dt, tag="iota_r")
        nc.vector.tensor_copy(iota_r[:], iota_i[:])
        # slot-id row replicated on all partitions [128, NC] (f32, exact)
        slotrow_i = wp.tile([128, NC], i32, tag="slotrow_i")
        nc.gpsimd.iota(slotrow_i[:], pattern=[[1, NC]], base=0, channel_multiplier=0)
        slotrow_f = wp.tile([128, NC], f32, tag="slotrow_f")
        nc.vector.tensor_copy(slotrow_f[:], slotrow_i[:])



        # token-major input rows -> base canvas to DRAM
        for tcn in range(8):
            t = sp.tile([128, 2 * DIM], f32, tag="ftmt", name="ftmt")
            nc.gpsimd.dma_start(t[:], ftm_d[tcn * 128 : (tcn + 1) * 128, :])
            bt = sp.tile([128, DIM], f32, tag="basetm", name="basetm")
            nc.gpsimd.tensor_tensor(bt[:], t[:, 0:DIM], t[:, DIM : 2 * DIM], op=OP.add)
            nc.sync.dma_start(out_d[tcn * 128 : (tcn + 1) * 128, :], bt[:])

        # =========== agent (fp32r matmuls, full N) ===========
        h_sb = []
        for oc in range(4):
            ps = pps.tile([128, N], f32, tag="sps")
            for hs in HN:
                for kc in range(4):
                    nc.tensor.matmul(
                        ps[:, hs],
                        aw1T_sb[kc][:, oc * 128 : (oc + 1) * 128],
                        xcat[kc][:, hs],
                        start=(kc == 0),
                        stop=(kc == 3),
                    )
            t = sp.tile([128, N], f32r_dt, tag="agbig", bufs=4, name=f"h{oc}")
            nc.scalar.activation(
                t[:], ps[:], AF.Relu,
                bias=agbi_sb[:, oc : oc + 1], scale=agsc_sb[:, oc : oc + 1],
            )
            h_sb.append(t)
        zps = ppm.tile([1, N], f32, tag="misc")
        for hs in HN:
            for oc in range(4):
                nc.tensor.matmul(
                    zps[0:1, hs], aw2c_sb[:, oc : oc + 1], h_sb[oc][:, hs],
                    start=(oc == 0), stop=(oc == 3),
                )
        w_row = xp.tile([1, N], f32, tag="w_row")
        nc.scalar.activation(w_row[:], zps[:], AF.Sigmoid, bias=scal_sb[0:1, 0:1], scale=1.0)
        s_row = xp.tile([1, N], f32, tag="s_row")
        nc.scalar.activation(s_row[:], w_row[:], AF.Abs, bias=scal_sb[0:1, 3:4], scale=1.0)
        # wsmeta rows to DRAM (token-major w/s)
        nc.sync.dma_start(wsm_d[:, 0:1], w_row[:])
        nc.sync.dma_start(wsm_d[:, 1:2], s_row[:])

        # ---- rank columns via fused compare+reduce ----
        s_col = xp.tile([128, 8], f32, tag="s_col")
        for t in range(8):
            tp = ppm.tile([128, 64], f32, tag="misc")
            nc.tensor.transpose(tp[:, 0:1], s_row[0:1, t * 128 : (t + 1) * 128], ident[0:1, 0:1])
            nc.vector.tensor_copy(s_col[:, t : t + 1], tp[:, 0:1])
        sbc_ps = pps.tile([128, N], f32, tag="sps")
        for hs in HN:
            nc.tensor.matmul(sbc_ps[:, hs], ones_row[:], s_row[0:1, hs],
                             start=True, stop=True)
        s_bc = xp.tile([128, N], f32, tag="s_bc")
        nc.scalar.copy(s_bc[:], sbc_ps[:])
        # rankc[p, t] = #{j: s[j] > s[t*128+p]} in one fused pass per tile:
        # even tiles on DVE (is_gt + free-reduce), odd tiles on ACT via the
        # sign trick (sum sign(s - pivot) = #gt - #lt = 2#gt - (N-1)).
        rankc = xp.tile([128, 8], f32, tag="rankc")
        nsc = xp.tile([128, 8], f32, tag="nsc")
        nc.vector.tensor_scalar(nsc[:], s_col[:], -1.0, None, op0=OP.mult)
        sacc = xp.tile([128, 8], f32, tag="sacc")
        junk = [
            sp.tile([128, N], bf16, tag="agbig", bufs=4, name="junkv"),
            sp.tile([128, N], bf16, tag="agbig", bufs=4, name="junkg"),
        ]
        for t in range(8):
            if t % 2 == 0:
                nc.vector.tensor_scalar(
                    junk[0][:], s_bc[:], s_col[:, t : t + 1], 0.0,
                    op0=OP.is_gt, op1=OP.add, accum_out=rankc[:, t : t + 1],
                )
            else:
                nc.scalar.activation(
                    junk[1][:], s_bc[:], AF.Sign,
                    bias=nsc[:, t : t + 1], scale=1.0,
                    accum_out=sacc[:, t : t + 1],
                )
        nc.vector.tensor_scalar(
            rankc[:, 1:8:2], sacc[:, 1:8:2], float(N - 1), 0.5,
            op0=OP.add, op1=OP.mult,
        )

        # ---- rank -> token inverse permutation, fully on-chip:
        # G_t[p, r] = (rank[t*128+p] == r); token id = iota_f[p, t] = 128t+p;
        # idx_row[r] = sum_t iota_t.T @ G_t (single fp32r matmul per tile).
        idxps = ppo.tile([1, NC], f32, tag="ops")
        for t in range(8):
            eng = nc.vector if t % 2 == 0 else nc.gpsimd
            G = sp.tile([128, NC], f32r_dt, tag="agbig", bufs=4, name="G")
            eng.tensor_scalar(G[:], slotrow_f[:], rankc[:, t : t + 1], None, op0=OP.is_equal)
            for hs in HC:
                nc.tensor.matmul(idxps[0:1, hs], iota_r[:, t : t + 1], G[:, hs],
                                 start=(t == 0), stop=(t == 7))
        idx_row = sp.tile([1, NC], f32, tag="idx_row")
        nc.vector.tensor_copy(idx_row[:], idxps[:])

        # ---- k-ratio MLP -> rank threshold (off critical path: only
        # slot masks / scatter redirect / attn bias need it) ----
        gsum = sp.tile([1, 1], f32, tag="tiny")
        nc.vector.reduce_sum(gsum[:], s_row[:], axis=AX.X)
        t16 = sp.tile([1, 16], f32, tag="t16")
        nc.vector.tensor_scalar(t16[:], hrow_sb[0:1, 0:16], gsum[0:1, 0:1], None, op0=OP.mult)
        nc.vector.tensor_tensor(t16[:], t16[:], hrow_sb[0:1, 16:32], op=OP.add)
        t16r = sp.tile([1, 16], f32, tag="t16r")
        nc.scalar.activation(t16r[:], t16[:], AF.Relu)
        t16j = sp.tile([1, 16], f32, tag="t16j")
        urow = sp.tile([1, 1], f32, tag="tiny2")
        nc.vector.scalar_tensor_tensor(
            t16j[:], t16r[:], 1.0, hrow_sb[0:1, 32:48],
            op0=OP.mult, op1=OP.mult, accum_out=urow[:],
        )
        thr = xp.tile([1, 1], f32, tag="thr")
        nc.scalar.activation(thr[:], urow[:], AF.Sigmoid, bias=scal_sb[0:1, 1:2], scale=1.0)
        nc.vector.tensor_scalar(thr[:], thr[:], 819.2, 102.4 - 1.0, op0=OP.mult, op1=OP.add)
        nc.vector.tensor_scalar(thr[:], thr[:], 63.0, None, op0=OP.max)
        # thr broadcast to a column [128, 1]
        thr_ps = ppm.tile([128, 64], f32, tag="misc")
        nc.tensor.matmul(thr_ps[:, 0:1], ones_row[:], thr[0:1, 0:1], start=True, stop=True)
        thr_col = xp.tile([128, 1], f32, tag="thr_col")
        nc.vector.tensor_copy(thr_col[:], thr_ps[:, 0:1])
        hsc_ps = ppm.tile([128, 64], f32, tag="misc")
        nc.tensor.matmul(hsc_ps[:, 0:1], ones_row[:], scal_sb[0:1, 2:3], start=True, stop=True)
        hs_col = xp.tile([128, 1], f32, tag="hs_col")
        nc.vector.tensor_copy(hs_col[:], hsc_ps[:, 0:1])
        idx_colf = sp.tile([128, nkc], f32, tag="idx_colf")
        for c in range(nkc):
            tp = ppm.tile([128, 64], f32, tag="misc")
            nc.tensor.transpose(tp[:, 0:1], idx_row[0:1, c * 128 : (c + 1) * 128], ident[0:1, 0:1])
            nc.vector.tensor_copy(idx_colf[:, c : c + 1], tp[:, 0:1])
        idx_col = xp.tile([128, nkc], i32, tag="idx_col")
        nc.vector.tensor_copy(idx_col[:], idx_colf[:])

        # slot mask columns (slot id <= thr) and attention pad bias
        slotm = xp.tile([128, nkc], f32, tag="slotm")
        nc.vector.tensor_scalar(slotm[:], iota_f[:, 0:nkc], thr_col[:, 0:1], None, op0=OP.is_le)
        mbias_c = xp.tile([128, nkc], f32, tag="mbias_c")
        nc.vector.tensor_scalar(mbias_c[:], slotm[:], 1.0, 1e9, op0=OP.subtract, op1=OP.mult)

        # ---- gather compact tokens + w/s ----
        Xc = {"ir": [], "vis": []}
        for c in range(2):
            Xc["ir"].append(xp.tile([128, NC], f32, tag=f"Xir{c}", name=f"Xir{c}"))
            Xc["vis"].append(xp.tile([128, NC], f32, tag=f"Xvis{c}", name=f"Xvis{c}"))
        base_tm = []
        ws_col = xp.tile([128, 2 * nkc], f32, tag="ws_col")
        for tcn in range(nkc):
            g = sp.tile([128, 2 * DIM], f32, tag="gtok", bufs=3)
            nc.gpsimd.indirect_dma_start(
                out=g[:],
                out_offset=None,
                in_=ftm_d[:],
                in_offset=bass.IndirectOffsetOnAxis(ap=idx_col[:, tcn : tcn + 1], axis=0),
            )
            bt = xp.tile([128, DIM], f32, tag=f"btm{tcn}", name=f"btm{tcn}")
            nc.gpsimd.tensor_tensor(bt[:], g[:, 0:DIM], g[:, DIM : 2 * DIM], op=OP.add)
            base_tm.append(bt)
            for c in range(2):
                tp = ppm.tile([128, 128], f32, tag="misc")
                nc.tensor.transpose(tp[:], g[:, c * 128 : (c + 1) * 128], ident[:])
                nc.vector.tensor_copy(Xc["ir"][c][:, tcn * 128 : (tcn + 1) * 128], tp[:])
                tp = ppm.tile([128, 128], f32, tag="misc")
                nc.tensor.transpose(tp[:], g[:, DIM + c * 128 : DIM + (c + 1) * 128], ident[:])
                nc.vector.tensor_copy(Xc["vis"][c][:, tcn * 128 : (tcn + 1) * 128], tp[:])

        # =========== compact mixers ===========
        def layernorm_bf(Xs, stage):
            xbf = []
            for c in range(2):
                t = sp.tile([128, NC], bf16, tag="xbf", bufs=3)
                nc.vector.tensor_copy(t[:], Xs[c][:])
                xbf.append(t)
            xsq = []
            for c in range(2):
                t = sp.tile([128, NC], bf16, tag="xsq", bufs=3)
                nc.gpsimd.tensor_tensor(t[:], xbf[c][:], xbf[c][:], op=OP.mult)
                xsq.append(t)
            st = ppm.tile([65, NC], f32, tag="misc")
            for hs in HC:
                for c in range(2):
                    nc.tensor.matmul(st[0:1, hs], ones_colb[:], xbf[c][:, hs],
                                     start=(c == 0), stop=(c == 1))
                for c in range(2):
                    nc.tensor.matmul(st[64:65, hs], ones_colb[:], xsq[c][:, hs],
                                     start=(c == 0), stop=(c == 1))
            mrow = sp.tile([1, NC], f32, tag="rowf", bufs=3, name="mrow")
            nc.vector.tensor_scalar(mrow[:], st[0:1, :], 1.0 / DIM, None, op0=OP.mult)
            m2row = sp.tile([1, NC], f32, tag="rowf", bufs=3, name="m2row")
            nc.vector.tensor_scalar(m2row[:], st[64:65, :], 1.0 / DIM, None, op0=OP.mult)
            vrow = sp.tile([1, NC], f32, tag="rowf", bufs=3, name="vrow")
            nc.vector.tensor_tensor(vrow[:], mrow[:], mrow[:], op=OP.mult)
            nc.vector.tensor_tensor(vrow[:], m2row[:], vrow[:], op=OP.subtract)
            sdrow = sp.tile([1, NC], f32, tag="rowf", bufs=3, name="sdrow")
            nc.scalar.activation(sdrow[:], vrow[:], AF.Ln, bias=scal_sb[0:1, 4:5], scale=1.0)
            irow = sp.tile([1, NC], f32, tag="rowf", bufs=3, name="irow")
            nc.scalar.activation(irow[:], sdrow[:], AF.Exp, scale=-0.5)
            mrow_bf = sp.tile([1, NC], bf16, tag="rowbf", bufs=2, name="mrow_bf")
            nc.vector.tensor_copy(mrow_bf[:], mrow[:])
            irow_bf = sp.tile([1, NC], bf16, tag="rowbf", bufs=2, name="irow_bf")
            nc.vector.tensor_copy(irow_bf[:], irow[:])
            mb_ps = pps.tile([128, NC], f32, tag="sps")
            ib_ps = pps.tile([128, NC], f32, tag="sps")
            for hs in HC:
                nc.tensor.matmul(mb_ps[:, hs], ones_row_bf[:], mrow_bf[0:1, hs], start=True, stop=True)
                nc.tensor.matmul(ib_ps[:, hs], ones_row_bf[:], irow_bf[0:1, hs], start=True, stop=True)
            xln = []
            for c in range(2):
                t = hp.tile([128, NC], bf16, tag=f"xln{stage}{c}")
                nc.vector.tensor_tensor(t[:], xbf[c][:], mb_ps[:], op=OP.subtract)
                nc.vector.tensor_tensor(t[:], t[:], ib_ps[:], op=OP.mult)
                xln.append(t)
            return xln

        mstate = {}

        def mixer_front(p):
            w = mix_w[p]
            Xs = Xc[p]
            xln = layernorm_bf(Xs, p + "a")
            qkv = []
            for oc in range(6):
                ps = pps.tile([128, NC], f32, tag="sps")
                for hs in HC:
                    for c in range(2):
                        nc.tensor.matmul(
                            ps[:, hs],
                            w["qkvT"][c][:, oc * 128 : (oc + 1) * 128],
                            xln[c][:, hs],
                            start=(c == 0), stop=(c == 1),
                        )
                t = hp.tile([128, NC], bf16, tag=f"{p}qkv{oc}")
                if oc < 2:
                    nc.vector.tensor_scalar(
                        t[:], ps[:], w["qb"][:, oc : oc + 1], None, op0=OP.add
                    )
                else:
                    nc.vector.tensor_copy(t[:], ps[:])
                qkv.append(t)
            mstate[p] = {"qkv": qkv}

        def mixer_attn(p):
            w = mix_w[p]
            Xs = Xc[p]
            qkv = mstate[p]["qkv"]
            o_sb = [
                hp.tile([128, NC], bf16, tag=p + "osb0", name="osb0"),
                hp.tile([128, NC], bf16, tag=p + "osb1", name="osb1"),
            ]
            for h in range(NH):
                hi = h % 2
                qt = qkv[h // 2][hi * 64 : hi * 64 + 64, :]
                kt = qkv[2 + h // 2][hi * 64 : hi * 64 + 64, :]
                vt = qkv[4 + h // 2][hi * 64 : hi * 64 + 64, :]
                ops = ppo.tile([65, NC], f32, tag="ops")
                for kc in range(nkc):
                    tp = ppm.tile([128, 64], bf16, tag="misc")
                    idnt = ident_bf[:] if hi == 0 else ident_bf_hi[64:128, 0:64]
                    nc.tensor.transpose(tp[:], vt[:, kc * 128 : (kc + 1) * 128], idnt)
                    vtm = sp.tile([128, 65], bf16, tag="vtm", bufs=6)
                    nc.vector.tensor_copy(vtm[:, 0:64], tp[:])
                    nc.vector.memset(vtm[:, 64:65], 1.0)
                    sps = pps.tile([128, NC], f32, tag="sps")
                    for hs in HC:
                        nc.tensor.matmul(
                            sps[:, hs], kt[:, kc * 128 : (kc + 1) * 128], qt[:, hs],
                            start=True, stop=True,
                        )
                    pt = sp.tile([128, NC], bf16, tag="pt", bufs=6)
                    nc.scalar.activation(
                        pt[:], sps[:], AF.Exp,
                        bias=mbias_c[:, kc : kc + 1], scale=0.125,
                    )
                    for hs in HC:
                        nc.tensor.matmul(
                            ops[:, hs], vtm[:], pt[:, hs],
                            start=(kc == 0), stop=(kc == nkc - 1),
                        )
                rrow = sp.tile([1, NC], f32, tag="rowf", bufs=3, name="rrow")
                nc.vector.reciprocal(rrow[:], ops[64:65, :])
                rrow_bf = sp.tile([1, NC], bf16, tag="rowbf", bufs=2, name="rrow_bf")
                nc.vector.tensor_copy(rrow_bf[:], rrow[:])
                rb_ps = ppm.tile([64, NC], f32, tag="misc")
                for hs in HC:
                    nc.tensor.matmul(
                        rb_ps[:, hs], ones_row_bf[0:1, 0:64], rrow_bf[0:1, hs],
                        start=True, stop=True,
                    )
                rb = sp.tile([64, NC], bf16, tag="rbs")
                nc.vector.tensor_copy(rb[:], rb_ps[:])
                dst = o_sb[h // 2][hi * 64 : hi * 64 + 64, :]
                nc.vector.tensor_tensor(dst, ops[0:64, :], rb[:], op=OP.mult)
            mstate[p]["osb"] = o_sb

        def mixer_proj(p):
            w = mix_w[p]
            Xs = Xc[p]
            o_sb = mstate[p]["osb"]
            for oc in range(2):
                ps = pps.tile([128, NC], f32, tag="sps")
                for hs in HC:
                    for c in range(2):
                        nc.tensor.matmul(
                            ps[:, hs],
                            w["outT"][c][:, oc * 128 : (oc + 1) * 128],
                            o_sb[c][:, hs],
                            start=(c == 0), stop=(c == 1),
                        )
                nc.vector.scalar_tensor_tensor(
                    Xs[oc][:], ps[:], w["outb"][:, oc : oc + 1], Xs[oc][:],
                    op0=OP.add, op1=OP.add,
                )

        def mixer_ffn(p):
            w = mix_w[p]
            Xs = Xc[p]
            xln2 = layernorm_bf(Xs, p + "b")
            hf = []
            for oc in range(8):
                ps = pps.tile([128, NC], f32, tag="sps")
                for hs in HC:
                    for c in range(2):
                        nc.tensor.matmul(
                            ps[:, hs],
                            w["w1T"][c][:, oc * 128 : (oc + 1) * 128],
                            xln2[c][:, hs],
                            start=(c == 0), stop=(c == 1),
                        )
                t = hp.tile([128, NC], bf16, tag=f"hf{oc}", bufs=2)
                nc.scalar.activation(t[:], ps[:], AF.Gelu, bias=w["b1"][:, oc : oc + 1], scale=1.0)
                hf.append(t)
            for oc in range(2):
                ps = pps.tile([128, NC], f32, tag="sps")
                for hs in HC:
                    for kc in range(8):
                        nc.tensor.matmul(
                            ps[:, hs],
                            w["w2T"][kc][:, oc * 128 : (oc + 1) * 128],
                            hf[kc][:, hs],
                            start=(kc == 0), stop=(kc == 7),
                        )
                nc.vector.scalar_tensor_tensor(
                    Xs[oc][:], ps[:], w["b2"][:, oc : oc + 1],
                    Xs[oc][:], op0=OP.add, op1=OP.add,
                )

        mixer_front("ir")
        mixer_front("vis")
        for tcn in range(nkc):
            gw = sp.tile([128, 2], f32, tag="gws", name="gws")
            nc.gpsimd.indirect_dma_start(
                out=gw[:],
                out_offset=None,
                in_=wsm_d[:],
                in_offset=bass.IndirectOffsetOnAxis(ap=idx_col[:, tcn : tcn + 1], axis=0),
            )
            nc.vector.tensor_copy(ws_col[:, 2 * tcn : 2 * tcn + 2], gw[:])
        mixer_attn("ir")
        # scatter offsets with trash redirect: 1024 + m*(idx-1024)
        idx_f = sp.tile([128, nkc], f32, tag="idx_f")
        nc.vector.tensor_scalar(idx_f[:], idx_colf[:], float(N), None, op0=OP.subtract)
        nc.vector.tensor_tensor(idx_f[:], idx_f[:], slotm[:], op=OP.mult)
        nc.vector.tensor_scalar(idx_f[:], idx_f[:], float(N), None, op0=OP.add)
        idx_scat = xp.tile([128, nkc], i32, tag="idx_scat")
        nc.vector.tensor_copy(idx_scat[:], idx_f[:])
        mixer_proj("ir")
        mixer_attn("vis")
        mixer_ffn("ir")
        mixer_proj("vis")
        mixer_ffn("vis")
        if debug:
            for c in range(2):
                nc.sync.dma_start(dbg_d["d_refir"][c * 128 : (c + 1) * 128, :], Xc["ir"][c][:])
                nc.sync.dma_start(dbg_d["d_refvis"][c * 128 : (c + 1) * 128, :], Xc["vis"][c][:])

        # =========== edge (token-major) + scatter ===========
        for tcn in range(nkc):
            csl = slice(tcn * 128, tcn * 128 + 128)
            irt, vist = [], []
            for c in range(2):
                tp = pps.tile([128, 128], f32, tag="sps")
                nc.tensor.transpose(tp[:], Xc["ir"][c][:, csl], ident[:])
                t = sp.tile([128, 128], f32, tag="irt", bufs=3)
                nc.vector.tensor_copy(t[:], tp[:])
                irt.append(t)
                tp = ppm.tile([128, 128], f32, tag="misc")
                nc.tensor.transpose(tp[:], Xc["vis"][c][:, csl], ident[:])
                t = sp.tile([128, 128], f32, tag="vist", bufs=3)
                nc.vector.tensor_copy(t[:], tp[:])
                vist.append(t)
            w_cs = ws_col[:, 2 * tcn : 2 * tcn + 1]
            s_cs = ws_col[:, 2 * tcn + 1 : 2 * tcn + 2]
            e_cs = sp.tile([128, 1], f32, tag="e_cs")
            nc.vector.tensor_scalar(e_cs[:], s_cs, hs_col[:, 0:1], 1.0, op0=OP.mult, op1=OP.add)
            ed = sp.tile([128, DIM], f32, tag="edge", bufs=3)
            for c in range(2):
                dsl = slice(c * 128, c * 128 + 128)
                nc.vector.tensor_tensor(ed[:, dsl], irt[c][:], vist[c][:], op=OP.subtract)
                nc.vector.tensor_scalar(ed[:, dsl], ed[:, dsl], w_cs, None, op0=OP.mult)
                nc.vector.tensor_tensor(ed[:, dsl], ed[:, dsl], vist[c][:], op=OP.add)
                nc.vector.tensor_tensor(
                    ed[:, dsl], ed[:, dsl], base_tm[tcn][:, dsl], op=OP.add
                )
                nc.vector.tensor_scalar(ed[:, dsl], ed[:, dsl], e_cs[:, 0:1], None, op0=OP.mult)
            nc.gpsimd.indirect_dma_start(
                out=out_d[:],
                out_offset=bass.IndirectOffsetOnAxis(ap=idx_scat[:, tcn : tcn + 1], axis=0),
                in_=ed[:, 0:DIM],
                in_offset=None,
            )
        ctx.close()
    _split_multi_waits(nc)
    return nc


def _get_nc(nkc, debug=False):
    key = ("nc2", nkc, debug)
    if key not in _CACHE:
        _install_compat()
        _CACHE[key] = _build(nkc, debug=debug)
    return _CACHE[key]


def kernel(**inputs):
    from concourse.bass_utils import run_bass_kernel_spmd

    kmax = _host_k_estimate(inputs)
    nkc = min(8, max(1, -(-(kmax + 2) // 128)))
    nc = _get_nc(nkc, debug=False)
    per_core = _prep_inputs(inputs)
    res = run_bass_kernel_spmd(nc, per_core, core_ids=list(range(B)))
    out = np.stack(
        [res.results[b]["out"][:N].T.copy() for b in range(B)], axis=0
    )
    return out.reshape(B, DIM, 32, 32).astype(np.float32)


def kernel_debug(**inputs):
    from concourse.bass_utils import run_bass_kernel_spmd

    kmax = _host_k_estimate(inputs)
    nkc = min(8, max(1, -(-(kmax + 2) // 128)))
    nc = _get_nc(nkc, debug=True)
    per_core = _prep_inputs(inputs)
    res = run_bass_kernel_spmd(nc, per_core, core_ids=list(range(B)))
    return res, nkc



# revision 18
# speedup vs baseline: 1.4394x; 1.4394x over previous
"""Trainium2 Bass kernel, v2: top-k COMPACTED mixers.

Same agent/rank machinery as v1 (full-N, fp32), but the mixers run on
a compacted token set: the top-k tokens (k per-sample dynamic, padded
to a compile-time multiple of 128 chosen from a cheap host estimate)
are gathered via indirect DMA using a device-computed rank->token
inverse permutation. Attention pads are masked with a slot-index bias;
the edge highlighting is evaluated in token-major form (per-token
scalars become per-partition scalars) and scattered back over a
token-major base canvas with out-of-quota slots redirected to a trash
row. The host transposes the token-major output back to (C, H, W).
"""

import numpy as np


DIM = 256
N = 1024
HID = 512
NH = 4
B = 8

_CACHE = {}


def _install_compat():
    """Environment shims: walrus here accepts at most ONE sync-wait per
    instruction; Tile's kernel-tail drain aggregates many -> split them
    onto single-wait DVE nops. Also make upload_artifacts local-only."""
    import concourse.mybir as mybir
    import concourse.tile as tile
    from concourse.vector_clock import ScopedClock
    from concourse import bass_utils

    if not getattr(tile.TileContext, "_drain_patched", False):

        def _patched(self, tick_clock, wait_clock):
            nc = self.nc
            drain_inst = nc.sync.drain()
            wait_clock.add_sem_waits(
                drain_inst.ins, ScopedClock({None: tick_clock.global_clock})
            )
            si = drain_inst.ins.sync_info
            waits = list(si.on_wait)
            if len(waits) > 1:
                drain_inst.ins.sync_info = mybir.SyncInfo(
                    on_wait=[], on_update=list(si.on_update)
                )
                for i in range(len(waits)):
                    nop = nc.vector.engine_nop()
                    nop.ins.sync_info = mybir.SyncInfo(
                        on_wait=waits[i : i + 1], on_update=[]
                    )
            nc.all_engine_barrier()
            assert self.sems is not None
            popped = nc._tile_sem_poison_stack.pop()
            assert popped is self._sem_poison
            nc.clear_and_free_semaphores(list(self.sems.allocated().values()))
            nc.all_engine_barrier()

        tile.TileContext._drain_and_barrier = _patched
        tile.TileContext._drain_patched = True

    bass_utils.upload_artifacts = lambda tmpdir: str(tmpdir)


def _to_bf16(a):
    import ml_dtypes

    return np.asarray(a, dtype=np.float32).astype(ml_dtypes.bfloat16)


def _prep_inputs_v1(inputs):
    """Host-side packing: per-core activation tensors + replicated
    (layout-transposed, LN-folded) weights."""
    f = {k: np.asarray(v, dtype=np.float32) for k, v in inputs.items()}
    shared = {}

    ascale = (f["bn_g"] / np.float32(np.sqrt(1.0 + 1e-5))).astype(np.float32)
    abias = (f["ab1"] * ascale + f["bn_b"]).astype(np.float32)
    shared["aw1T"] = np.ascontiguousarray(f["aw1"].T).astype(np.float32)
    shared["agsc"] = ascale
    shared["agbi"] = abias
    shared["aw2T"] = np.ascontiguousarray(f["aw2"].reshape(1, HID).T).reshape(HID)
    scal = np.zeros((1, 8), np.float32)
    scal[0, 0] = f["ab2"].reshape(-1)[0]
    scal[0, 1] = f["hb2"].reshape(-1)[0]
    scal[0, 2] = np.float32(f["highlight_scale"])
    scal[0, 3] = np.float32(-0.5)
    scal[0, 4] = np.float32(1e-5)
    shared["scal"] = scal
    hrow = np.zeros((1, 48), np.float32)
    hrow[0, 0:16] = f["hw1"].reshape(16) / np.float32(N)
    hrow[0, 16:32] = f["hb1"].reshape(16)
    hrow[0, 32:48] = f["hw2"].reshape(16)
    shared["hrow"] = hrow

    for p in ("ir", "vis"):
        ln_g = f[p + "_ln_g"]
        ln_b = f[p + "_ln_b"]
        qkv_w = f[p + "_qkv_w"]
        qkv_b = f[p + "_qkv_b"]
        out_w = f[p + "_out_w"]
        out_b = f[p + "_out_b"]
        w1 = f[p + "_ffn_w1"]
        b1 = f[p + "_ffn_b1"]
        w2 = f[p + "_ffn_w2"]
        b2 = f[p + "_ffn_b2"]
        qkvT_eff = (qkv_w * ln_g[None, :]).T  # [256, 768]
        qkvb_eff = qkv_b + qkv_w @ ln_b
        w1T_eff = (w1 * ln_g[None, :]).T  # [256, 1024]
        b1_eff = b1 + w1 @ ln_b
        bv = qkvb_eff[2 * DIM :]
        outb_eff = out_b + out_w @ bv
        shared[p + "_qkvT"] = _to_bf16(qkvT_eff)
        shared[p + "_qb"] = qkvb_eff[:DIM].astype(np.float32)
        shared[p + "_outT"] = _to_bf16(out_w.T)
        shared[p + "_outb"] = outb_eff.astype(np.float32)
        shared[p + "_w1T"] = _to_bf16(w1T_eff)
        shared[p + "_b1"] = b1_eff.astype(np.float32)
        shared[p + "_w2T"] = _to_bf16(w2.T)
        shared[p + "_b2"] = b2.astype(np.float32)

    per_core = []
    fir = f["f_ir"].reshape(B, DIM, N)
    fvis = f["f_vis"].reshape(B, DIM, N)
    for b in range(B):
        m = dict(shared)
        m["fir"] = np.ascontiguousarray(fir[b])
        m["fvis"] = np.ascontiguousarray(fvis[b])
        per_core.append(m)
    return per_core




def _split_multi_waits(nc):
    """This container's walrus accepts only ONE sync-wait per
    instruction: hoist extra waits onto same-engine nop carriers
    inserted immediately before the instruction."""
    import concourse.mybir as mybir

    for f in nc.m.functions:
        for bb in f.blocks:
            insts = list(bb.instructions)
            rebuilt = []
            changed = False
            for inst in insts:
                si = inst.sync_info
                waits = list(si.on_wait) if si is not None else []
                if len(waits) > 1:
                    changed = True
                    eng = inst.engine
                    for wx in waits[:-1]:
                        wrap = nc.engines[eng].nop(nofuse=True)
                        mi = wrap.ins
                        # remove from wherever add_instruction appended it
                        for f2 in nc.m.functions:
                            for bb2 in f2.blocks:
                                lst = list(bb2.instructions)
                                if lst and lst[-1] is mi:
                                    lst.pop()
                                    bb2.instructions = lst
                        mi.sync_info = mybir.SyncInfo(on_wait=[wx], on_update=[])
                        rebuilt.append(mi)
                    inst.sync_info = mybir.SyncInfo(
                        on_wait=[waits[-1]], on_update=list(si.on_update)
                    )
                rebuilt.append(inst)
            if changed:
                bb.instructions = rebuilt




def _host_k_estimate(inputs):
    f = {k: np.asarray(v, dtype=np.float32) for k, v in inputs.items()}
    x = np.concatenate([f["f_ir"], f["f_vis"]], axis=1).reshape(B, 2 * DIM, N)
    h = np.einsum("bcn,oc->bon", x, f["aw1"]) + f["ab1"][None, :, None]
    h = h / np.float32(np.sqrt(1.0 + 1e-5)) * f["bn_g"][None, :, None] + f["bn_b"][None, :, None]
    h = np.maximum(h, 0)
    z = np.einsum("bcn,oc->bon", h, f["aw2"]) + f["ab2"][None, :, None]
    w = 1.0 / (1.0 + np.exp(-z))
    score = np.abs(w - 0.5).reshape(B, N)
    gs = score.mean(axis=1, keepdims=True)
    t = np.maximum(gs @ f["hw1"].T + f["hb1"], 0)
    kr = 1.0 / (1.0 + np.exp(-(t @ f["hw2"].T + f["hb2"]))) * 0.8 + 0.1
    k = np.maximum(np.floor(N * kr[:, 0]).astype(np.int64), 64)
    return int(k.max())


def _prep_inputs(inputs):
    per_core = _prep_inputs_v1(inputs)
    f_ir = np.asarray(inputs["f_ir"], np.float32).reshape(B, DIM, N)
    f_vis = np.asarray(inputs["f_vis"], np.float32).reshape(B, DIM, N)
    for b in range(B):
        ftm = np.empty((N, 2 * DIM), np.float32)
        ftm[:, :DIM] = f_ir[b].T
        ftm[:, DIM:] = f_vis[b].T
        per_core[b]["ftm"] = ftm
    return per_core


def _build(nkc, debug=False):
    from contextlib import ExitStack

    import concourse.bass as bass
    import concourse.mybir as mybir
    import concourse.tile as tile
    from concourse.masks import make_identity

    f32 = mybir.dt.float32
    bf16 = mybir.dt.bfloat16
    i32 = mybir.dt.int32
    AF = mybir.ActivationFunctionType
    OP = mybir.AluOpType
    AX = mybir.AxisListType

    NC = nkc * 128  # compact token count (padded)

    def halves(n):
        out = []
        o = 0
        while o < n:
            w = min(512, n - o)
            out.append(slice(o, o + w))
            o += w
        return out

    HN = halves(N)
    HC = halves(NC)

    nc = bass.Bass("TRN2", target_bir_lowering=False, debug=False, enable_asserts=True)

    def din(name, shape, dt=f32):
        return nc.dram_tensor(name, shape, dt, kind="ExternalInput").ap()

    f32r_dt = mybir.dt.float32r
    fir_d = din("fir", [DIM, N], f32r_dt)
    fvis_d = din("fvis", [DIM, N], f32r_dt)
    ftm_d = din("ftm", [N, 2 * DIM])
    aw1T_d = din("aw1T", [2 * DIM, HID], f32r_dt)
    agsc_d = din("agsc", [HID])
    agbi_d = din("agbi", [HID])
    aw2T_d = din("aw2T", [HID], f32r_dt)
    scal_d = din("scal", [1, 8])
    hrow_d = din("hrow", [1, 48])
    mix_d = {}
    for p in ("ir", "vis"):
        mix_d[p] = dict(
            qkvT=din(p + "_qkvT", [DIM, 3 * DIM], bf16),
            qb=din(p + "_qb", [DIM]),
            outT=din(p + "_outT", [DIM, DIM], bf16),
            outb=din(p + "_outb", [DIM]),
            w1T=din(p + "_w1T", [DIM, 4 * DIM], bf16),
            b1=din(p + "_b1", [4 * DIM]),
            w2T=din(p + "_w2T", [4 * DIM, DIM], bf16),
            b2=din(p + "_b2", [DIM]),
        )

    out_d = nc.dram_tensor("out", [N + 1, DIM], f32, kind="ExternalOutput").ap()
    wsm_d = nc.dram_tensor("wsmeta", [N, 2], f32, kind="Internal").ap()
    dbg_d = {}
    if debug:
        for nm, shp in (
            ("d_mask", [1, N]),
            ("d_idx", [128, 8]),
            ("d_refir", [DIM, NC]),
            ("d_refvis", [DIM, NC]),
        ):
            dbg_d[nm] = nc.dram_tensor(nm, shp, f32, kind="ExternalOutput").ap()

    ctx = ExitStack()
    with tile.TileContext(nc) as tc:
        wp = ctx.enter_context(tc.tile_pool(name="wp", bufs=1))
        xp = ctx.enter_context(tc.tile_pool(name="xp", bufs=1))
        hp = ctx.enter_context(tc.tile_pool(name="hp", bufs=1))
        sp = ctx.enter_context(tc.tile_pool(name="sp", bufs=2))
        pps = ctx.enter_context(tc.tile_pool(name="pps", bufs=2, space="PSUM"))
        ppo = ctx.enter_context(tc.tile_pool(name="ppo", bufs=1, space="PSUM"))
        ppm = ctx.enter_context(tc.tile_pool(name="ppm", bufs=1, space="PSUM"))

        def col128(dram_vec, n, dt=f32):
            t = wp.tile([128, n], dt, tag=dram_vec.tensor.name)
            nc.gpsimd.dma_start(t[:], dram_vec.rearrange("(c p) -> p c", p=128))
            return t

        # ---- inputs first (agent path is the serial head of the kernel) ----
        fir_sb, fvis_sb = [], []
        for c in range(2):
            t = xp.tile([128, N], f32r_dt, tag=f"fir{c}", name=f"fir{c}")
            nc.sync.dma_start(t[:], fir_d[c * 128 : (c + 1) * 128, :])
            fir_sb.append(t)
            t = xp.tile([128, N], f32r_dt, tag=f"fvis{c}", name=f"fvis{c}")
            nc.sync.dma_start(t[:], fvis_d[c * 128 : (c + 1) * 128, :])
            fvis_sb.append(t)
        xcat = fir_sb + fvis_sb

        # ---- weights ----
        aw1T_sb = []
        for kc in range(4):
            t = wp.tile([128, HID], f32r_dt, tag=f"aw1T{kc}")
            nc.gpsimd.dma_start(t[:], aw1T_d[kc * 128 : (kc + 1) * 128, :])
            aw1T_sb.append(t)
        agsc_sb = col128(agsc_d, 4)
        agbi_sb = col128(agbi_d, 4)
        aw2c_sb = col128(aw2T_d, 4, f32r_dt)
        scal_sb = wp.tile([1, 8], f32, tag="scal")
        nc.gpsimd.dma_start(scal_sb[:], scal_d[:])
        hrow_sb = wp.tile([1, 48], f32, tag="hrow")
        nc.gpsimd.dma_start(hrow_sb[:], hrow_d[:])

        mix_w = {}
        for p in ("ir", "vis"):
            d = mix_d[p]
            w = {"qkvT": [], "outT": [], "w1T": [], "w2T": []}
            for c in range(2):
                t = wp.tile([128, 3 * DIM], bf16, tag=f"{p}qkvT{c}")
                nc.gpsimd.dma_start(t[:], d["qkvT"][c * 128 : (c + 1) * 128, :])
                w["qkvT"].append(t)
            w["qb"] = col128(d["qb"], 2)
            for c in range(2):
                t = wp.tile([128, DIM], bf16, tag=f"{p}outT{c}")
                nc.gpsimd.dma_start(t[:], d["outT"][c * 128 : (c + 1) * 128, :])
                w["outT"].append(t)
            w["outb"] = col128(d["outb"], 2)
            for c in range(2):
                t = wp.tile([128, 4 * DIM], bf16, tag=f"{p}w1T{c}")
                nc.sync.dma_start(t[:], d["w1T"][c * 128 : (c + 1) * 128, :])
                w["w1T"].append(t)
            w["b1"] = col128(d["b1"], 8)
            for kc in range(8):
                t = wp.tile([128, DIM], bf16, tag=f"{p}w2T{kc}")
                nc.sync.dma_start(t[:], d["w2T"][kc * 128 : (kc + 1) * 128, :])
                w["w2T"].append(t)
            w["b2"] = col128(d["b2"], 2)
            mix_w[p] = w

        ones_row = wp.tile([1, 128], f32, tag="ones_row")
        nc.vector.memset(ones_row[:], 1.0)
        ones_row_bf = wp.tile([1, 128], bf16, tag="ones_row_bf")
        nc.vector.memset(ones_row_bf[:], 1.0)
        ones_colb = wp.tile([128, 1], bf16, tag="ones_colb")
        nc.vector.memset(ones_colb[:], 1.0)
        ident = wp.tile([128, 128], f32, tag="ident")
        make_identity(nc, ident[:])
        ident_bf = wp.tile([64, 64], bf16, tag="ident_bf")
        make_identity(nc, ident_bf[:])
        ident_bf_hi = wp.tile([128, 64], bf16, tag="ident_bf_hi")
        nc.gpsimd.memset(ident_bf_hi[:], 0.0)
        make_identity(nc, ident_bf_hi[64:128, 0:64], nomemset=True)
        # slot iota [128, 8]: val[p, c] = c*128 + p
        iota_i = wp.tile([128, 8], i32, tag="iota_i")
        nc.gpsimd.iota(iota_i[:], pattern=[[128, 8]], base=0, channel_multiplier=1)
        iota_f = wp.tile([128, 8], f32, tag="iota_f")
        nc.vector.tensor_copy(iota_f[:], iota_i[:])
        iota_r = wp.tile([128, 8], f32r_dt, tag="iota_r")
        nc.vector.tensor_copy(iota_r[:], iota_i[:])
        # slot-id row replicated on all partitions [128, NC] (f32, exact)
        slotrow_i = wp.tile([128, NC], i32, tag="slotrow_i")
        nc.gpsimd.iota(slotrow_i[:], pattern=[[1, NC]], base=0, channel_multiplier=0)
        slotrow_f = wp.tile([128, NC], f32, tag="slotrow_f")
        nc.vector.tensor_copy(slotrow_f[:], slotrow_i[:])



        # token-major input rows -> base canvas to DRAM
        for tcn in range(8):
            t = sp.tile([128, 2 * DIM], f32, tag="ftmt", name="ftmt")
            nc.gpsimd.dma_start(t[:], ftm_d[tcn * 128 : (tcn + 1) * 128, :])
            bt = sp.tile([128, DIM], f32, tag="basetm", name="basetm")
            nc.gpsimd.tensor_tensor(bt[:], t[:, 0:DIM], t[:, DIM : 2 * DIM], op=OP.add)
            nc.sync.dma_start(out_d[tcn * 128 : (tcn + 1) * 128, :], bt[:])

        # =========== agent (fp32r matmuls, full N) ===========
        h_sb = []
        for oc in range(4):
            ps = pps.tile([128, N], f32, tag="sps")
            for hs in HN:
                for kc in range(4):
                    nc.tensor.matmul(
                        ps[:, hs],
                        aw1T_sb[kc][:, oc * 128 : (oc + 1) * 128],
                        xcat[kc][:, hs],
                        start=(kc == 0),
                        stop=(kc == 3),
                    )
            t = sp.tile([128, N], f32r_dt, tag="agbig", bufs=4, name=f"h{oc}")
            nc.scalar.activation(
                t[:], ps[:], AF.Relu,
                bias=agbi_sb[:, oc : oc + 1], scale=agsc_sb[:, oc : oc + 1],
            )
            h_sb.append(t)
        zps = ppm.tile([1, N], f32, tag="misc")
        for hs in HN:
            for oc in range(4):
                nc.tensor.matmul(
                    zps[0:1, hs], aw2c_sb[:, oc : oc + 1], h_sb[oc][:, hs],
                    start=(oc == 0), stop=(oc == 3),
                )
        w_row = xp.tile([1, N], f32, tag="w_row")
        nc.scalar.activation(w_row[:], zps[:], AF.Sigmoid, bias=scal_sb[0:1, 0:1], scale=1.0)
        s_row = xp.tile([1, N], f32, tag="s_row")
        nc.scalar.activation(s_row[:], w_row[:], AF.Abs, bias=scal_sb[0:1, 3:4], scale=1.0)
        # wsmeta rows to DRAM (token-major w/s)
        nc.sync.dma_start(wsm_d[:, 0:1], w_row[:])
        nc.sync.dma_start(wsm_d[:, 1:2], s_row[:])

        # ---- rank columns via fused compare+reduce ----
        s_col = xp.tile([128, 8], f32, tag="s_col")
        for t in range(8):
            tp = ppm.tile([128, 64], f32, tag="misc")
            nc.tensor.transpose(tp[:, 0:1], s_row[0:1, t * 128 : (t + 1) * 128], ident[0:1, 0:1])
            nc.vector.tensor_copy(s_col[:, t : t + 1], tp[:, 0:1])
        sbc_ps = pps.tile([128, N], f32, tag="sps")
        for hs in HN:
            nc.tensor.matmul(sbc_ps[:, hs], ones_row[:], s_row[0:1, hs],
                             start=True, stop=True)
        s_bc = xp.tile([128, N], f32, tag="s_bc")
        nc.scalar.copy(s_bc[:], sbc_ps[:])
        # rankc[p, t] = #{j: s[j] > s[t*128+p]} in one fused pass per tile:
        # even tiles on DVE (is_gt + free-reduce), odd tiles on ACT via the
        # sign trick (sum sign(s - pivot) = #gt - #lt = 2#gt - (N-1)).
        rankc = xp.tile([128, 8], f32, tag="rankc")
        nsc = xp.tile([128, 8], f32, tag="nsc")
        nc.vector.tensor_scalar(nsc[:], s_col[:], -1.0, None, op0=OP.mult)
        sacc = xp.tile([128, 8], f32, tag="sacc")
        junk = [
            sp.tile([128, N], bf16, tag="agbig", bufs=4, name="junkv"),
            sp.tile([128, N], bf16, tag="agbig", bufs=4, name="junkg"),
        ]
        for t in range(8):
            if t % 2 == 0:
                nc.vector.tensor_scalar(
                    junk[0][:], s_bc[:], s_col[:, t : t + 1], 0.0,
                    op0=OP.is_gt, op1=OP.add, accum_out=rankc[:, t : t + 1],
                )
            else:
                nc.scalar.activation(
                    junk[1][:], s_bc[:], AF.Sign,
                    bias=nsc[:, t : t + 1], scale=1.0,
                    accum_out=sacc[:, t : t + 1],
                )
        nc.vector.tensor_scalar(
            rankc[:, 1:8:2], sacc[:, 1:8:2], float(N - 1), 0.5,
            op0=OP.add, op1=OP.mult,
        )

        # ---- rank -> token inverse permutation, fully on-chip:
        # G_t[p, r] = (rank[t*128+p] == r); token id = iota_f[p, t] = 128t+p;
        # idx_row[r] = sum_t iota_t.T @ G_t (single fp32r matmul per tile).
        idxps = ppo.tile([1, NC], f32, tag="ops")
        for t in range(8):
            G = sp.tile([128, NC], f32r_dt, tag="agbig", bufs=4, name="G")
            nc.vector.tensor_scalar(G[:], slotrow_f[:], rankc[:, t : t + 1], None, op0=OP.is_equal)
            for hs in HC:
                nc.tensor.matmul(idxps[0:1, hs], iota_r[:, t : t + 1], G[:, hs],
                                 start=(t == 0), stop=(t == 7))
        idx_row = sp.tile([1, NC], f32, tag="idx_row")
        nc.vector.tensor_copy(idx_row[:], idxps[:])

        # ---- k-ratio MLP -> rank threshold (off critical path: only
        # slot masks / scatter redirect / attn bias need it) ----
        gsum = sp.tile([1, 1], f32, tag="tiny")
        nc.vector.reduce_sum(gsum[:], s_row[:], axis=AX.X)
        t16 = sp.tile([1, 16], f32, tag="t16")
        nc.vector.tensor_scalar(t16[:], hrow_sb[0:1, 0:16], gsum[0:1, 0:1], None, op0=OP.mult)
        nc.vector.tensor_tensor(t16[:], t16[:], hrow_sb[0:1, 16:32], op=OP.add)
        t16r = sp.tile([1, 16], f32, tag="t16r")
        nc.scalar.activation(t16r[:], t16[:], AF.Relu)
        t16j = sp.tile([1, 16], f32, tag="t16j")
        urow = sp.tile([1, 1], f32, tag="tiny2")
        nc.vector.scalar_tensor_tensor(
            t16j[:], t16r[:], 1.0, hrow_sb[0:1, 32:48],
            op0=OP.mult, op1=OP.mult, accum_out=urow[:],
        )
        thr = xp.tile([1, 1], f32, tag="thr")
        nc.scalar.activation(thr[:], urow[:], AF.Sigmoid, bias=scal_sb[0:1, 1:2], scale=1.0)
        nc.vector.tensor_scalar(thr[:], thr[:], 819.2, 102.4 - 1.0, op0=OP.mult, op1=OP.add)
        nc.vector.tensor_scalar(thr[:], thr[:], 63.0, None, op0=OP.max)
        # thr broadcast to a column [128, 1]
        thr_ps = ppm.tile([128, 64], f32, tag="misc")
        nc.tensor.matmul(thr_ps[:, 0:1], ones_row[:], thr[0:1, 0:1], start=True, stop=True)
        thr_col = xp.tile([128, 1], f32, tag="thr_col")
        nc.vector.tensor_copy(thr_col[:], thr_ps[:, 0:1])
        hsc_ps = ppm.tile([128, 64], f32, tag="misc")
        nc.tensor.matmul(hsc_ps[:, 0:1], ones_row[:], scal_sb[0:1, 2:3], start=True, stop=True)
        hs_col = xp.tile([128, 1], f32, tag="hs_col")
        nc.vector.tensor_copy(hs_col[:], hsc_ps[:, 0:1])
        idx_colf = sp.tile([128, nkc], f32, tag="idx_colf")
        for c in range(nkc):
            tp = ppm.tile([128, 64], f32, tag="misc")
            nc.tensor.transpose(tp[:, 0:1], idx_row[0:1, c * 128 : (c + 1) * 128], ident[0:1, 0:1])
            nc.vector.tensor_copy(idx_colf[:, c : c + 1], tp[:, 0:1])
        idx_col = xp.tile([128, nkc], i32, tag="idx_col")
        nc.vector.tensor_copy(idx_col[:], idx_colf[:])

        # slot mask columns (slot id <= thr) and attention pad bias
        slotm = xp.tile([128, nkc], f32, tag="slotm")
        nc.vector.tensor_scalar(slotm[:], iota_f[:, 0:nkc], thr_col[:, 0:1], None, op0=OP.is_le)
        mbias_c = xp.tile([128, nkc], f32, tag="mbias_c")
        nc.vector.tensor_scalar(mbias_c[:], slotm[:], 1.0, 1e9, op0=OP.subtract, op1=OP.mult)

        # ---- gather compact tokens + w/s ----
        Xc = {"ir": [], "vis": []}
        for c in range(2):
            Xc["ir"].append(xp.tile([128, NC], f32, tag=f"Xir{c}", name=f"Xir{c}"))
            Xc["vis"].append(xp.tile([128, NC], f32, tag=f"Xvis{c}", name=f"Xvis{c}"))
        base_tm = []
        ws_col = xp.tile([128, 2 * nkc], f32, tag="ws_col")
        for tcn in range(nkc):
            g = sp.tile([128, 2 * DIM], f32, tag="gtok", bufs=3)
            nc.gpsimd.indirect_dma_start(
                out=g[:],
                out_offset=None,
                in_=ftm_d[:],
                in_offset=bass.IndirectOffsetOnAxis(ap=idx_col[:, tcn : tcn + 1], axis=0),
            )
            bt = xp.tile([128, DIM], f32, tag=f"btm{tcn}", name=f"btm{tcn}")
            nc.gpsimd.tensor_tensor(bt[:], g[:, 0:DIM], g[:, DIM : 2 * DIM], op=OP.add)
            base_tm.append(bt)
            for c in range(2):
                tp = ppm.tile([128, 128], f32, tag="misc")
                nc.tensor.transpose(tp[:], g[:, c * 128 : (c + 1) * 128], ident[:])
                nc.vector.tensor_copy(Xc["ir"][c][:, tcn * 128 : (tcn + 1) * 128], tp[:])
                tp = ppm.tile([128, 128], f32, tag="misc")
                nc.tensor.transpose(tp[:], g[:, DIM + c * 128 : DIM + (c + 1) * 128], ident[:])
                nc.vector.tensor_copy(Xc["vis"][c][:, tcn * 128 : (tcn + 1) * 128], tp[:])

        # =========== compact mixers ===========
        def layernorm_bf(Xs, stage):
            xbf = []
            for c in range(2):
                t = sp.tile([128, NC], bf16, tag="xbf", bufs=3)
                nc.vector.tensor_copy(t[:], Xs[c][:])
                xbf.append(t)
            xsq = []
            for c in range(2):
                t = sp.tile([128, NC], bf16, tag="xsq", bufs=3)
                nc.gpsimd.tensor_tensor(t[:], xbf[c][:], xbf[c][:], op=OP.mult)
                xsq.append(t)
            st = ppm.tile([65, NC], f32, tag="misc")
            for hs in HC:
                for c in range(2):
                    nc.tensor.matmul(st[0:1, hs], ones_colb[:], xbf[c][:, hs],
                                     start=(c == 0), stop=(c == 1))
                for c in range(2):
                    nc.tensor.matmul(st[64:65, hs], ones_colb[:], xsq[c][:, hs],
                                     start=(c == 0), stop=(c == 1))
            mrow = sp.tile([1, NC], f32, tag="rowf", bufs=3, name="mrow")
            nc.vector.tensor_scalar(mrow[:], st[0:1, :], 1.0 / DIM, None, op0=OP.mult)
            m2row = sp.tile([1, NC], f32, tag="rowf", bufs=3, name="m2row")
            nc.vector.tensor_scalar(m2row[:], st[64:65, :], 1.0 / DIM, None, op0=OP.mult)
            vrow = sp.tile([1, NC], f32, tag="rowf", bufs=3, name="vrow")
            nc.vector.tensor_tensor(vrow[:], mrow[:], mrow[:], op=OP.mult)
            nc.vector.tensor_tensor(vrow[:], m2row[:], vrow[:], op=OP.subtract)
            sdrow = sp.tile([1, NC], f32, tag="rowf", bufs=3, name="sdrow")
            nc.scalar.activation(sdrow[:], vrow[:], AF.Ln, bias=scal_sb[0:1, 4:5], scale=1.0)
            irow = sp.tile([1, NC], f32, tag="rowf", bufs=3, name="irow")
            nc.scalar.activation(irow[:], sdrow[:], AF.Exp, scale=-0.5)
            mrow_bf = sp.tile([1, NC], bf16, tag="rowbf", bufs=2, name="mrow_bf")
            nc.vector.tensor_copy(mrow_bf[:], mrow[:])
            irow_bf = sp.tile([1, NC], bf16, tag="rowbf", bufs=2, name="irow_bf")
            nc.vector.tensor_copy(irow_bf[:], irow[:])
            mb_ps = pps.tile([128, NC], f32, tag="sps")
            ib_ps = pps.tile([128, NC], f32, tag="sps")
            for hs in HC:
                nc.tensor.matmul(mb_ps[:, hs], ones_row_bf[:], mrow_bf[0:1, hs], start=True, stop=True)
                nc.tensor.matmul(ib_ps[:, hs], ones_row_bf[:], irow_bf[0:1, hs], start=True, stop=True)
            xln = []
            for c in range(2):
                t = hp.tile([128, NC], bf16, tag=f"xln{stage}{c}")
                nc.vector.tensor_tensor(t[:], xbf[c][:], mb_ps[:], op=OP.subtract)
                nc.vector.tensor_tensor(t[:], t[:], ib_ps[:], op=OP.mult)
                xln.append(t)
            return xln

        mstate = {}

        def mixer_front(p):
            w = mix_w[p]
            Xs = Xc[p]
            xln = layernorm_bf(Xs, p + "a")
            qkv = []
            for oc in range(6):
                ps = pps.tile([128, NC], f32, tag="sps")
                for hs in HC:
                    for c in range(2):
                        nc.tensor.matmul(
                            ps[:, hs],
                            w["qkvT"][c][:, oc * 128 : (oc + 1) * 128],
                            xln[c][:, hs],
                            start=(c == 0), stop=(c == 1),
                        )
                t = hp.tile([128, NC], bf16, tag=f"{p}qkv{oc}")
                if oc < 2:
                    nc.vector.tensor_scalar(
                        t[:], ps[:], w["qb"][:, oc : oc + 1], None, op0=OP.add
                    )
                else:
                    nc.vector.tensor_copy(t[:], ps[:])
                qkv.append(t)
            mstate[p] = {"qkv": qkv}

        def mixer_attn(p):
            w = mix_w[p]
            Xs = Xc[p]
            qkv = mstate[p]["qkv"]
            o_sb = [
                hp.tile([128, NC], bf16, tag=p + "osb0", name="osb0"),
                hp.tile([128, NC], bf16, tag=p + "osb1", name="osb1"),
            ]
            for h in range(NH):
                hi = h % 2
                qt = qkv[h // 2][hi * 64 : hi * 64 + 64, :]
                kt = qkv[2 + h // 2][hi * 64 : hi * 64 + 64, :]
                vt = qkv[4 + h // 2][hi * 64 : hi * 64 + 64, :]
                ops = ppo.tile([65, NC], f32, tag="ops")
                for kc in range(nkc):
                    tp = ppm.tile([128, 64], bf16, tag="misc")
                    idnt = ident_bf[:] if hi == 0 else ident_bf_hi[64:128, 0:64]
                    nc.tensor.transpose(tp[:], vt[:, kc * 128 : (kc + 1) * 128], idnt)
                    vtm = sp.tile([128, 65], bf16, tag="vtm", bufs=6)
                    nc.vector.tensor_copy(vtm[:, 0:64], tp[:])
                    nc.vector.memset(vtm[:, 64:65], 1.0)
                    sps = pps.tile([128, NC], f32, tag="sps")
                    for hs in HC:
                        nc.tensor.matmul(
                            sps[:, hs], kt[:, kc * 128 : (kc + 1) * 128], qt[:, hs],
                            start=True, stop=True,
                        )
                    pt = sp.tile([128, NC], bf16, tag="pt", bufs=6)
                    nc.scalar.activation(
                        pt[:], sps[:], AF.Exp,
                        bias=mbias_c[:, kc : kc + 1], scale=0.125,
                    )
                    for hs in HC:
                        nc.tensor.matmul(
                            ops[:, hs], vtm[:], pt[:, hs],
                            start=(kc == 0), stop=(kc == nkc - 1),
                        )
                lrow = sp.tile([1, NC], f32, tag="rowf", bufs=3, name="lrow")
                nc.scalar.activation(lrow[:], ops[64:65, :], AF.Ln)
                rrow = sp.tile([1, NC], f32, tag="rowf", bufs=3, name="rrow")
                nc.scalar.activation(rrow[:], lrow[:], AF.Exp, scale=-1.0)
                rrow_bf = sp.tile([1, NC], bf16, tag="rowbf", bufs=2, name="rrow_bf")
                nc.vector.tensor_copy(rrow_bf[:], rrow[:])
                rb_ps = ppm.tile([64, NC], f32, tag="misc")
                for hs in HC:
                    nc.tensor.matmul(
                        rb_ps[:, hs], ones_row_bf[0:1, 0:64], rrow_bf[0:1, hs],
                        start=True, stop=True,
                    )
                rb = sp.tile([64, NC], bf16, tag="rbs")
                nc.vector.tensor_copy(rb[:], rb_ps[:])
                dst = o_sb[h // 2][hi * 64 : hi * 64 + 64, :]
                nc.vector.tensor_tensor(dst, ops[0:64, :], rb[:], op=OP.mult)
            mstate[p]["osb"] = o_sb

        def mixer_proj(p):
            w = mix_w[p]
            Xs = Xc[p]
            o_sb = mstate[p]["osb"]
            for oc in range(2):
                ps = pps.tile([128, NC], f32, tag="sps")
                for hs in HC:
                    for c in range(2):
                        nc.tensor.matmul(
                            ps[:, hs],
                            w["outT"][c][:, oc * 128 : (oc + 1) * 128],
                            o_sb[c][:, hs],
                            start=(c == 0), stop=(c == 1),
                        )
                nc.vector.scalar_tensor_tensor(
                    Xs[oc][:], ps[:], w["outb"][:, oc : oc + 1], Xs[oc][:],
                    op0=OP.add, op1=OP.add,
                )

        def mixer_ffn(p):
            w = mix_w[p]
            Xs = Xc[p]
            xln2 = layernorm_bf(Xs, p + "b")
            hf = []
            for oc in range(8):
                ps = pps.tile([128, NC], f32, tag="sps")
                for hs in HC:
                    for c in range(2):
                        nc.tensor.matmul(
                            ps[:, hs],
                            w["w1T"][c][:, oc * 128 : (oc + 1) * 128],
                            xln2[c][:, hs],
                            start=(c == 0), stop=(c == 1),
                        )
                t = hp.tile([128, NC], bf16, tag=f"hf{oc}", bufs=2)
                nc.scalar.activation(t[:], ps[:], AF.Gelu, bias=w["b1"][:, oc : oc + 1], scale=1.0)
                hf.append(t)
            for oc in range(2):
                ps = pps.tile([128, NC], f32, tag="sps")
                for hs in HC:
                    for kc in range(8):
                        nc.tensor.matmul(
                            ps[:, hs],
                            w["w2T"][kc][:, oc * 128 : (oc + 1) * 128],
                            hf[kc][:, hs],
                            start=(kc == 0), stop=(kc == 7),
                        )
                nc.vector.scalar_tensor_tensor(
                    Xs[oc][:], ps[:], w["b2"][:, oc : oc + 1],
                    Xs[oc][:], op0=OP.add, op1=OP.add,
                )

        mixer_front("ir")
        mixer_front("vis")
        for tcn in range(nkc):
            gw = sp.tile([128, 2], f32, tag="gws", name="gws")
            nc.gpsimd.indirect_dma_start(
                out=gw[:],
                out_offset=None,
                in_=wsm_d[:],
                in_offset=bass.IndirectOffsetOnAxis(ap=idx_col[:, tcn : tcn + 1], axis=0),
            )
            nc.vector.tensor_copy(ws_col[:, 2 * tcn : 2 * tcn + 2], gw[:])
        mixer_attn("ir")
        # scatter offsets with trash redirect: 1024 + m*(idx-1024)
        idx_f = sp.tile([128, nkc], f32, tag="idx_f")
        nc.vector.tensor_scalar(idx_f[:], idx_colf[:], float(N), None, op0=OP.subtract)
        nc.vector.tensor_tensor(idx_f[:], idx_f[:], slotm[:], op=OP.mult)
        nc.vector.tensor_scalar(idx_f[:], idx_f[:], float(N), None, op0=OP.add)
        idx_scat = xp.tile([128, nkc], i32, tag="idx_scat")
        nc.vector.tensor_copy(idx_scat[:], idx_f[:])
        mixer_proj("ir")
        mixer_attn("vis")
        mixer_ffn("ir")
        mixer_proj("vis")
        mixer_ffn("vis")
        if debug:
            for c in range(2):
                nc.sync.dma_start(dbg_d["d_refir"][c * 128 : (c + 1) * 128, :], Xc["ir"][c][:])
                nc.sync.dma_start(dbg_d["d_refvis"][c * 128 : (c + 1) * 128, :], Xc["vis"][c][:])

        # =========== edge (token-major) + scatter ===========
        for tcn in range(nkc):
            csl = slice(tcn * 128, tcn * 128 + 128)
            irt, vist = [], []
            for c in range(2):
                tp = pps.tile([128, 128], f32, tag="sps")
                nc.tensor.transpose(tp[:], Xc["ir"][c][:, csl], ident[:])
                t = sp.tile([128, 128], f32, tag="irt", bufs=3)
                nc.vector.tensor_copy(t[:], tp[:])
                irt.append(t)
                tp = ppm.tile([128, 128], f32, tag="misc")
                nc.tensor.transpose(tp[:], Xc["vis"][c][:, csl], ident[:])
                t = sp.tile([128, 128], f32, tag="vist", bufs=3)
                nc.vector.tensor_copy(t[:], tp[:])
                vist.append(t)
            w_cs = ws_col[:, 2 * tcn : 2 * tcn + 1]
            s_cs = ws_col[:, 2 * tcn + 1 : 2 * tcn + 2]
            e_cs = sp.tile([128, 1], f32, tag="e_cs")
            nc.vector.tensor_scalar(e_cs[:], s_cs, hs_col[:, 0:1], 1.0, op0=OP.mult, op1=OP.add)
            ed = sp.tile([128, DIM], f32, tag="edge", bufs=3)
            for c in range(2):
                dsl = slice(c * 128, c * 128 + 128)
                nc.vector.tensor_tensor(ed[:, dsl], irt[c][:], vist[c][:], op=OP.subtract)
                nc.vector.tensor_scalar(ed[:, dsl], ed[:, dsl], w_cs, None, op0=OP.mult)
                nc.vector.tensor_tensor(ed[:, dsl], ed[:, dsl], vist[c][:], op=OP.add)
                nc.vector.tensor_tensor(
                    ed[:, dsl], ed[:, dsl], base_tm[tcn][:, dsl], op=OP.add
                )
                nc.vector.tensor_scalar(ed[:, dsl], ed[:, dsl], e_cs[:, 0:1], None, op0=OP.mult)
            nc.gpsimd.indirect_dma_start(
                out=out_d[:],
                out_offset=bass.IndirectOffsetOnAxis(ap=idx_scat[:, tcn : tcn + 1], axis=0),
                in_=ed[:, 0:DIM],
                in_offset=None,
            )
        ctx.close()
    _split_multi_waits(nc)
    return nc


def _get_nc(nkc, debug=False):
    key = ("nc2", nkc, debug)
    if key not in _CACHE:
        _install_compat()
        _CACHE[key] = _build(nkc, debug=debug)
    return _CACHE[key]


def kernel(**inputs):
    from concourse.bass_utils import run_bass_kernel_spmd

    kmax = _host_k_estimate(inputs)
    nkc = min(8, max(1, -(-(kmax + 2) // 128)))
    nc = _get_nc(nkc, debug=False)
    per_core = _prep_inputs(inputs)
    res = run_bass_kernel_spmd(nc, per_core, core_ids=list(range(B)))
    out = np.stack(
        [res.results[b]["out"][:N].T.copy() for b in range(B)], axis=0
    )
    return out.reshape(B, DIM, 32, 32).astype(np.float32)


def kernel_debug(**inputs):
    from concourse.bass_utils import run_bass_kernel_spmd

    kmax = _host_k_estimate(inputs)
    nkc = min(8, max(1, -(-(kmax + 2) // 128)))
    nc = _get_nc(nkc, debug=True)
    per_core = _prep_inputs(inputs)
    res = run_bass_kernel_spmd(nc, per_core, core_ids=list(range(B)))
    return res, nkc

